# revision 29
# baseline (speedup 1.0000x reference)
"""Trainium2 Bass kernel for the binarized 2-layer MLP (nn_FC_small).

Network (reference semantics):
    h  = sign(x) @ sign(W1).T            # [B, 512], B = 65536, in = 768
    h  = batchnorm(h, g1, b1)            # training-mode, full-batch stats
    h  = clip(h, -1, 1)                  # hardtanh (sign-preserving)
    o  = sign(h) @ sign(W2).T            # [B, 10]
    o  = batchnorm(o, g2, b2)

Key identities:
  * sign(clip(z)) == sign(z); with g>0: sign(BN(h)) == sign(h - T),
    T = mu - b*sd/g   (all in "psum units" -- see below).
  * Codes: {0,1} via DVE is_gt or +-1 via ACT sign.  A {0,1}-coded
    contraction half contributes sum(w*sx) + C where C = sum(sign(w)) is a
    per-output-column constant; C is absorbed by the BN mean (threshold for
    BN1, exact cancellation for BN2).

Performance structure (per core, batch 8192):
  - x streamed as RAW f32 (cast-DMA measured ~1.6x slower).
  - fp8 codes computed in natural layout with byte-strided outs so that
    byte-pairs (feat f, feat f+384) form uint16 words; PE-transposes run on
    the uint16 view: half the transpose count, and the transposed pair word
    directly provides the DoubleRow (Ko=2) operand pair.
  - Evac PSUM->SBUF as uint16 copies (DVE 2x mode), one op per half-chunk.
  - mm1: fp8 DoubleRow, k-outer/bc-inner so stationary reloads amortize.
  - BN stats as SUM/SUMSQ, combined across cores with AllReduce(add)
    (cheaper than AllGather + local 8-way reduce).
  - Phase B: s8 codes mixed DVE({0,1})/ACT(+-1) with per-hid-chunk weight
    scaling; mm2 bf16; mm2 evac via DMA (PSUM->SBUF), stats on DVE.

Sharding: data-parallel over batch across 8 cores (8192 rows each).
Output returned transposed [10, 8192] per core; host concatenates + .T.
"""
import numpy as np

import concourse.bass as bass
import concourse.bacc as bacc
import concourse.tile as tile
import concourse.mybir as mybir
from concourse import bass_utils
from concourse.masks import make_identity

F32 = mybir.dt.float32
F16 = mybir.dt.float16
BF16 = mybir.dt.bfloat16
FP8 = mybir.dt.float8e4
U16 = mybir.dt.uint16
GT = mybir.AluOpType.is_gt
MUL = mybir.AluOpType.mult
ADD = mybir.AluOpType.add
SUB = mybir.AluOpType.subtract
DR = mybir.MatmulPerfMode.DoubleRow
IDENT = mybir.ActivationFunctionType.Identity
SIGN = mybir.ActivationFunctionType.Sign

N_CORES = 8
B = 65536
IND, HID, OUT = 768, 512, 10
B_LOC = B // N_CORES          # 8192
BC = 512                      # batch chunk
NBC = B_LOC // BC             # 16
NT = 3                        # pair-plane tiles (384 pairs = 3*128)
HC = HID // 128               # 4 hid-chunks
EPS = 1e-5

_cache = {}


def build():
    if "nc" in _cache:
        return _cache["nc"]
    nc = bacc.Bacc("TRN2", target_bir_lowering=False, debug=False,
                   num_devices=N_CORES)
    x = nc.dram_tensor("x", [B_LOC, IND], F32, kind="ExternalInput")
    w1 = nc.dram_tensor("w1", [HID, IND], F32, kind="ExternalInput")
    w2 = nc.dram_tensor("w2", [OUT, HID], F32, kind="ExternalInput")
    g1 = nc.dram_tensor("g1", [HID], F32, kind="ExternalInput")
    b1 = nc.dram_tensor("b1", [HID], F32, kind="ExternalInput")
    g2 = nc.dram_tensor("g2", [OUT], F32, kind="ExternalInput")
    b2 = nc.dram_tensor("b2", [OUT], F32, kind="ExternalInput")
    o_out = nc.dram_tensor("o_out", [OUT, B_LOC], F32, kind="ExternalOutput")

    with tile.TileContext(nc) as tc:
        with (
            tc.tile_pool(name="cst", bufs=1) as cst,
            tc.tile_pool(name="stage", bufs=2) as stage,
            tc.tile_pool(name="wpool", bufs=1) as wpool,
            tc.tile_pool(name="code", bufs=2) as codep,
            tc.tile_pool(name="s8p", bufs=2) as s8p,
            tc.tile_pool(name="pst", bufs=2, space="PSUM") as pst,    # transposes
            tc.tile_pool(name="psm", bufs=3, space="PSUM") as psm,    # mm1/mm2
            tc.tile_pool(name="dram", bufs=1, space="DRAM") as dpool,
        ):
            ident = cst.tile([128, 128], BF16)
            make_identity(nc, ident[:])
            bias_m1 = cst.tile([128, 1], F32)
            nc.vector.memset(bias_m1[:], -1.0)
            bias_m2 = cst.tile([128, 1], F32)
            nc.vector.memset(bias_m2[:], -2.0)
            wup_sb = cst.tile([128, 1], F32)

            # prefetch first three x chunks (raw f32) ahead of everything;
            # the warm-up CC doorbell below blocks the gpsimd queue for a
            # while, so these must be enqueued first.
            xcs = {}
            for pc in range(2):
                xt = stage.tile([128, 4, IND], F32, tag="xc")
                nc.gpsimd.dma_start(
                    out=xt[:],
                    in_=x.ap()[pc * BC:(pc + 1) * BC].rearrange("(s p) f -> p s f", p=128))
                xcs[pc] = xt

            # ---------------- weight prep ----------------
            # W1 [512, 768] -> codes -> pair-layout fp8 w1sT[ki, t, ko, hid]
            #   ko=0: feats 128t+ki   as +-2 (pairs {0,1}-coded x)
            #   ko=1: feats 384+128t+ki as +-1 (pairs +-1-coded x)
            w1c = wpool.tile([128, 4, IND], F32)
            nc.sync.dma_start(out=w1c[:], in_=w1.ap().rearrange("(c p) f -> p c f", p=128))
            w1b = wpool.tile([128, 4, IND], BF16)
            nc.vector.tensor_scalar(w1b[:], w1c[:], 0.0, None, GT)
            w1sT = cst.tile([128, NT, 2, HID], FP8)   # 3 KB/part
            for k in range(6):
                t, ko = (k, 0) if k < 3 else (k - 3, 1)
                sc, bi = (4.0, bias_m2) if k < 3 else (2.0, bias_m1)
                pw = psm.tile([128, HC, 128], BF16, tag="mm")
                for c in range(HC):
                    nc.tensor.transpose(pw[:, c, :], w1b[:, c, k * 128:(k + 1) * 128], ident[:])
                nc.scalar.activation(w1sT[:, t, ko, :], pw[:].rearrange("p c f -> p (c f)"),
                                     IDENT, bias=bi[:], scale=sc)

            # ---------------- persistent buffers ----------------
            xT8 = cst.tile([128, NT, B_LOC], U16, tag="bigx")   # 48 KB/part
            h1s = cst.tile([128, HC, B_LOC], F16, tag="bigh")   # 64 KB/part
            st1 = cst.tile([128, HC, NBC * 6], F32)             # bn_stats per bc
            h2T = cst.tile([OUT, B_LOC], F32, tag="bigx")       # reuses xT8 slab
            st2 = cst.tile([OUT, NBC * 6], F32)

            # warm-up collective (pays ncfw cold-start during phase A).
            # The chunk prefetch above is already enqueued on the gpsimd
            # queue, so the doorbell stall overlaps chunk 0-2 compute.
            wloc = dpool.tile([128, 1], F32)
            wgat = dpool.tile([128 * N_CORES, 1], F32)
            with tc.high_priority():
                nc.vector.memset(wup_sb[:], 0.0)
                nc.sync.dma_start(out=wloc[:], in_=wup_sb[:])
                nc.gpsimd.collective_compute(
                    "AllGather", mybir.AluOpType.bypass,
                    ins=[wloc.opt()], outs=[wgat.opt()],
                    replica_groups=[list(range(N_CORES))])

            # ---------------- phase A ----------------
            # per chunk: DMA f32 -> codes (byte-strided fp8) -> bf16 pair
            # transposes -> evac into xT8.  mm1 h-blocks of the PREVIOUS
            # group are emitted between transpose halves so real-matmul
            # activity never pauses long enough for the PE HAM clock gate
            # to re-throttle.
            def mm_h_block(g, h):
                bs0 = 2 * g * BC
                mp = psm.tile([128, 2, BC], F32, tag="mm")
                for t in range(NT):
                    lw = w1sT[:, t, :, h * 128:(h + 1) * 128]
                    for j in range(2):
                        rhs = xT8[:, t, bs0 + j * BC: bs0 + (j + 1) * BC] \
                            .bitcast(FP8).rearrange("p (n j) -> p j n", j=2)
                        nc.tensor.matmul(
                            mp[:, j, :], lw, rhs,
                            start=(t == 0), stop=(t == NT - 1),
                            perf_mode=DR)
                mpw = mp[:].rearrange("p j n -> p (j n)")
                nc.scalar.copy(h1s[:, h, bs0:bs0 + 2 * BC], mpw)
                for j in range(2):
                    bc2 = 2 * g + j
                    nc.vector.bn_stats(st1[:, h, bc2 * 6:(bc2 + 1) * 6], mp[:, j, :])

            with nc.named_scope("phaseA"):
                for grp in range(NBC // 2):
                    hb = 0
                    for half2 in range(2):
                        bc = 2 * grp + half2
                        bs = bc * BC
                        if bc in xcs:
                            xc = xcs.pop(bc)
                        else:
                            xc = stage.tile([128, 4, IND], F32, tag="xc")
                            nc.gpsimd.dma_start(
                                out=xc[:],
                                in_=x.ap()[bs:bs + BC].rearrange("(s p) f -> p s f", p=128))
                        # codes in natural layout; byte j of pair p is feat
                        # p + 384*j.  cd viewed [128, 4s, 384 pairs, 2 bytes].
                        cd = codep.tile([128, 4, IND], FP8, tag="cd")
                        cdv = cd[:].rearrange("p s (q j) -> p s q j", j=2)
                        nc.vector.tensor_scalar(
                            cdv[:, :, :, 0], xc[:, :, 0:384], 0.0, None, GT)
                        nc.scalar.sign(cdv[:, :, :, 1], xc[:, :, 384:768])
                        for half in range(2):
                            tp = pst.tile([128, NT, 2, 128], BF16, tag="tp")
                            for t in range(NT):
                                for s2 in range(2):
                                    s = 2 * half + s2
                                    cdu = cd[:, s, :].bitcast(BF16)
                                    nc.tensor.transpose(
                                        tp[:, t, s2, :],
                                        cdu[:, t * 128:(t + 1) * 128],
                                        ident[:])
                            # one evac per half-chunk: [128, 3, 2, 128] bf16 view
                            ev_out = xT8[:, :, bs + half * 256: bs + half * 256 + 256] \
                                .rearrange("p t (s b) -> p t s b", s=2).bitcast(BF16)
                            ev_in = tp[:]
                            if half == 0:
                                nc.vector.tensor_copy(ev_out, ev_in)
                            else:
                                nc.scalar.copy(ev_out, ev_in)
                            if grp > 0:
                                mm_h_block(grp - 1, hb)
                                hb += 1
            # ---------------- local stats -> AllGather ----------------
            # agg1 [128, HC, 2] = (mean, var) over local batch; per-h aggr
            # is emitted right after that h's final stats so the collective
            # launch chain is as short as possible.
            agg1 = cst.tile([128, HC, 2], F32)
            loc1 = dpool.tile([128, HC * 2], F32)
            gat1 = dpool.tile([128 * N_CORES, HC * 2], F32)
            ga1 = cst.tile([128, N_CORES, HC * 2], F32)
            with nc.named_scope("phaseA"):
                for h in range(HC):
                    mm_h_block(NBC // 2 - 1, h)
                    with tc.high_priority():
                        nc.vector.bn_aggr(agg1[:, h, :],
                                          st1[:, h, :].rearrange("p (n s) -> p n s", s=6))
            with tc.high_priority():
                nc.gpsimd.dma_start(out=loc1[:], in_=agg1[:].rearrange("p c s -> p (c s)"))
                nc.gpsimd.collective_compute(
                    "AllGather", mybir.AluOpType.bypass,
                    ins=[loc1.opt()], outs=[gat1.opt()],
                    replica_groups=[list(range(N_CORES))])
                nc.sync.dma_start(out=ga1[:], in_=gat1[:].rearrange("(c p) s -> p c s", p=128))

            # W2 prep + g/b vectors (overlaps phase A / collective)
            w2n = cst.tile([OUT, HID], F32)
            nc.sync.dma_start(out=w2n[:], in_=w2.ap())
            w2b = cst.tile([OUT, HID], BF16)
            nc.vector.tensor_scalar(w2b[:], w2n[:], 0.0, None, GT)
            # hid-chunks 0..2 pair with {0,1}-coded s (DVE): w' = +-2
            # hid-chunk  3 pairs with +-1-coded s (ACT):     w' = +-1
            w2sT = cst.tile([128, HC, OUT], BF16)
            for c in range(HC):
                pw2 = psm.tile([128, OUT], BF16, tag="mm")
                nc.tensor.transpose(pw2[:], w2b[:, c * 128:(c + 1) * 128], ident[:OUT, :OUT])
                sc, bi = (4.0, bias_m2) if c < 3 else (2.0, bias_m1)
                nc.scalar.activation(w2sT[:, c, :], pw2[:], IDENT, bias=bi[:], scale=sc)
            g1c = cst.tile([128, HC], F32)
            b1c = cst.tile([128, HC], F32)
            for c in range(HC):
                nc.sync.dma_start(out=g1c[:, c:c + 1], in_=g1.ap()[c * 128:(c + 1) * 128])
                nc.sync.dma_start(out=b1c[:, c:c + 1], in_=b1.ap()[c * 128:(c + 1) * 128])
            g2c = cst.tile([OUT, 1], F32)
            b2c = cst.tile([OUT, 1], F32)
            nc.sync.dma_start(out=g2c[:], in_=g2.ap())
            nc.sync.dma_start(out=b2c[:], in_=b2.ap())

            # combine: mean_tot = avg(mean_c); var_tot = avg(var_c + mean_c^2) - mean_tot^2
            with nc.named_scope("combine1"):
                q1 = cst.tile([128, N_CORES, HC * 2], F32)
                nc.vector.tensor_tensor(q1[:], ga1[:], ga1[:], MUL)
                msum = cst.tile([128, HC * 2], F32)
                qsum = cst.tile([128, HC * 2], F32)
                nc.vector.tensor_reduce(msum[:], ga1[:].rearrange("p c s -> p s c"),
                                        mybir.AxisListType.X, ADD)
                nc.vector.tensor_reduce(qsum[:], q1[:].rearrange("p c s -> p s c"),
                                        mybir.AxisListType.X, ADD)
                m1 = cst.tile([128, HC], F32)
                mview = msum[:].rearrange("p (c s) -> p c s", s=2)
                qview = qsum[:].rearrange("p (c s) -> p c s", s=2)
                nc.vector.tensor_scalar(m1[:], mview[:, :, 0], 1.0 / N_CORES, None, MUL)
                e2 = cst.tile([128, HC], F32)
                nc.vector.tensor_tensor(e2[:], qview[:, :, 0], mview[:, :, 1], ADD)
                nc.vector.tensor_scalar(e2[:], e2[:], 1.0 / N_CORES, None, MUL)
                m1sq = cst.tile([128, HC], F32)
                nc.vector.tensor_tensor(m1sq[:], m1[:], m1[:], MUL)
                v1 = cst.tile([128, HC], F32)
                nc.vector.tensor_tensor(v1[:], e2[:], m1sq[:], SUB)
                sd1 = cst.tile([128, HC], F32)
                nc.vector.tensor_scalar(sd1[:], v1[:], 1.0, EPS, MUL, ADD)
                nc.scalar.sqrt(sd1[:], sd1[:])
                ig1 = cst.tile([128, HC], F32)
                nc.vector.reciprocal(ig1[:], g1c[:])
                corr = cst.tile([128, HC], F32)
                nc.vector.tensor_tensor(corr[:], b1c[:], ig1[:], MUL)
                nc.vector.tensor_tensor(corr[:], corr[:], sd1[:], MUL)
                posT = cst.tile([128, HC], F32)   # threshold for is_gt
                negT = cst.tile([128, HC], F32)   # -threshold for ACT Sign bias
                nc.vector.tensor_tensor(posT[:], m1[:], corr[:], SUB)
                nc.vector.tensor_scalar(negT[:], posT[:], -1.0, None, MUL)

            # ---------------- phase B ----------------
            with nc.named_scope("phaseB"):
                SLAB = 2048
                NSL = B_LOC // SLAB            # 4 slabs of 4 bc
                for sl in range(NSL):
                    ss = sl * SLAB
                    s8t = []
                    for h in range(HC):
                        s8 = s8p.tile([128, SLAB], BF16, tag=f"s8{h}")
                        s8t.append(s8)
                        if h < 3:
                            nc.vector.tensor_scalar(
                                s8[:], h1s[:, h, ss:ss + SLAB],
                                posT[:, h:h + 1], None, GT)
                        else:
                            nc.scalar.activation(
                                s8[:], h1s[:, h, ss:ss + SLAB],
                                SIGN, bias=negT[:, h:h + 1], scale=1.0)
                    for j in range(4):
                        bc = sl * 4 + j
                        bs = bc * BC
                        mp2 = psm.tile([OUT, BC], F32, tag="mm")
                        for h in range(HC):
                            nc.tensor.matmul(
                                mp2[:], w2sT[:, h, :], s8t[h][:, j * BC:(j + 1) * BC],
                                start=(h == 0), stop=(h == HC - 1))
                        nc.scalar.copy(h2T[:, bs:bs + BC], mp2[:])
                        nc.vector.bn_stats(st2[:, bc * 6:(bc + 1) * 6], mp2[:])

            agg2 = cst.tile([OUT, 2], F32)
            loc2 = dpool.tile([OUT, 2], F32)
            gat2 = dpool.tile([OUT * N_CORES, 2], F32)
            ga2 = cst.tile([OUT, N_CORES, 2], F32)
            with tc.high_priority():
                nc.vector.bn_aggr(agg2[:], st2[:].rearrange("p (n s) -> p n s", s=6))
                nc.gpsimd.dma_start(out=loc2[:], in_=agg2[:])
                nc.gpsimd.collective_compute(
                    "AllGather", mybir.AluOpType.bypass,
                    ins=[loc2.opt()], outs=[gat2.opt()],
                    replica_groups=[list(range(N_CORES))])
                nc.sync.dma_start(out=ga2[:], in_=gat2[:].rearrange("(c p) s -> p c s", p=OUT))

            with nc.named_scope("combine2"):
                q2 = cst.tile([OUT, N_CORES, 2], F32)
                nc.vector.tensor_tensor(q2[:], ga2[:], ga2[:], MUL)
                msum2 = cst.tile([OUT, 2], F32)
                qsum2 = cst.tile([OUT, 2], F32)
                nc.vector.tensor_reduce(msum2[:], ga2[:].rearrange("p c s -> p s c"),
                                        mybir.AxisListType.X, ADD)
                nc.vector.tensor_reduce(qsum2[:], q2[:].rearrange("p c s -> p s c"),
                                        mybir.AxisListType.X, ADD)
                m2 = cst.tile([OUT, 1], F32)
                nc.vector.tensor_scalar(m2[:], msum2[:, 0:1], 1.0 / N_CORES, None, MUL)
                e22 = cst.tile([OUT, 1], F32)
                nc.vector.tensor_tensor(e22[:], qsum2[:, 0:1], msum2[:, 1:2], ADD)
                nc.vector.tensor_scalar(e22[:], e22[:], 1.0 / N_CORES, None, MUL)
                m2sq = cst.tile([OUT, 1], F32)
                nc.vector.tensor_tensor(m2sq[:], m2[:], m2[:], MUL)
                v2 = cst.tile([OUT, 1], F32)
                nc.vector.tensor_tensor(v2[:], e22[:], m2sq[:], SUB)
                sd2 = cst.tile([OUT, 1], F32)
                nc.vector.tensor_scalar(sd2[:], v2[:], 1.0, EPS, MUL, ADD)
                nc.scalar.sqrt(sd2[:], sd2[:])
                r2 = cst.tile([OUT, 1], F32)
                nc.vector.reciprocal(r2[:], sd2[:])
                scale2 = cst.tile([OUT, 1], F32)
                nc.vector.tensor_tensor(scale2[:], r2[:], g2c[:], MUL)
                shift2 = cst.tile([OUT, 1], F32)
                nc.vector.tensor_tensor(shift2[:], m2[:], scale2[:], MUL)
                nc.vector.tensor_tensor(shift2[:], b2c[:], shift2[:], SUB)

            for sl in range(4):
                ss = sl * (B_LOC // 4)
                se = ss + B_LOC // 4
                if sl % 2 == 0:
                    nc.vector.tensor_scalar(h2T[:, ss:se], h2T[:, ss:se],
                                            scale2[:], shift2[:], MUL, ADD)
                else:
                    nc.scalar.activation(h2T[:, ss:se], h2T[:, ss:se],
                                         IDENT, bias=shift2[:], scale=scale2[:])
                nc.sync.dma_start(out=o_out.ap()[:, ss:se], in_=h2T[:, ss:se])

    nc.compile()
    _cache["nc"] = nc
    return nc


def kernel(x, W1, W2, g1, b1, g2, b2, _trace=False):
    nc = build()
    x = np.ascontiguousarray(np.asarray(x, dtype=np.float32))
    in_maps = []
    for c in range(N_CORES):
        in_maps.append({
            "x": x[c * B_LOC:(c + 1) * B_LOC],
            "w1": np.asarray(W1, np.float32),
            "w2": np.asarray(W2, np.float32),
            "g1": np.asarray(g1, np.float32),
            "b1": np.asarray(b1, np.float32),
            "g2": np.asarray(g2, np.float32),
            "b2": np.asarray(b2, np.float32),
        })
    res = bass_utils.run_bass_kernel_spmd(nc, in_maps, core_ids=list(range(N_CORES)),
                                          trace=_trace)
    out = np.concatenate([np.ascontiguousarray(r["o_out"].T) for r in res.results], axis=0)
    if _trace:
        kernel.last_results = res
    return out
